# revision 38
# baseline (speedup 1.0000x reference)
"""Trainium2 Bass kernel for nn_ModelNew_3556232921999.

Pipeline: ConvTranspose3d(16->32, k=3, s=2, p=1, op=1) -> MaxPool3d(2)
          -> softmax(ch) -> subtract -> swish -> max(ch)

Key algebraic structure exploited:
  * convT(stride 2) + maxpool(2,2) => output spatial == input spatial, and the
    pool window {2m, 2m+1}^3 corresponds to the 8 parity classes of the convT.
    Each parity class is a small conv over x with taps at offsets {0,1}^3.
    pooled[c, m] = max over the 8 classes (+ bias, which commutes with max).
  * All parity classes for one position come out of matmuls with
      lhsT = x-stack block [K=128 = (od,oh,ow,cin), M=128 positions]
    so channel/class dims land on the FREE axis and the softmax reductions
    are free-dim reductions.
  * Pair-max via ReLU identity: max(a,b) = b + relu(a-b). The pd-pair of
    classes is computed as D = x@(W0-W1) (one matmul), relu'd IN PLACE in
    PSUM by the scalar engine (PE's has_written bits survive engine writes),
    then B = x@W1 is accumulated on top with start=False. One segmented
    tensor_reduce(max) over the remaining 4 (ph,pw) candidates then yields
    the pooled value -- no DVE max tree.
  * swish/silu is quasiconvex => max_c silu(v_c) = max(silu(max_c v),
    silu(min_c v)), so silu runs on 2 values per position, not 32.

Sharding: data-parallel over batch B=16 -> 2 per core x 8 cores.
"""

import os
import sys

sys.path.insert(0, "/opt/trn_rl_repo")

import numpy as np
import ml_dtypes

# ---------------------------------------------------------------- constants
IN_C, OUT_C, K, STRIDE, PAD, OUT_PAD = 16, 32, 3, 2, 1, 1
B, D, H, W = 16, 16, 64, 64
N_CORES = 8
B_PER_CORE = B // N_CORES  # 2

PLANE = H * W            # 4096 positions per (b, d) plane
BLK = 128                # positions per matmul block
BLKS_PER_PLANE = PLANE // BLK      # 32
GRP = 8                  # matmul blocks per psum group (2-bank tile)
GRPS_PER_PLANE = BLKS_PER_PLANE // GRP  # 2

X_NP_DT = ml_dtypes.bfloat16   # x-stack storage dtype (DMA volume)
W_NP_DT = ml_dtypes.bfloat16   # conv weight dtype

_COMPILED = {}


def _tap(o, p):
    """Kernel tap index used by parity class p at window offset o, or None."""
    if p == 0:
        return 1 if o == 0 else None
    return 2 if o == 0 else 0


def build_w8(weight):
    """[128 rows=(od,oh,ow,cin), 2,2,2,32 cols=(pd,ph,pw,c)] conv matrix."""
    wr = np.zeros((2, 2, 2, IN_C, 2, 2, 2, OUT_C), dtype=np.float32)
    for od in range(2):
        for oh in range(2):
            for ow in range(2):
                for pd in range(2):
                    kd = _tap(od, pd)
                    if kd is None:
                        continue
                    for ph in range(2):
                        kh = _tap(oh, ph)
                        if kh is None:
                            continue
                        for pw in range(2):
                            kw = _tap(ow, pw)
                            if kw is None:
                                continue
                            # weight: [cin, cout, kd, kh, kw]
                            wr[od, oh, ow, :, pd, ph, pw, :] = weight[:, :, kd, kh, kw]
    return wr.reshape(128, 2, 2, 2, OUT_C)


def build_wrhs(weight):
    """[128, 256] = [D-half | B-half], col within half = (c, j=(ph,pw)).

    D = W_pd0 - W_pd1 (pair differences), B = W_pd1 (pair base), so that
    pairmax = B + relu(D)."""
    w8 = build_w8(weight)                      # [128, pd, ph, pw, c]
    # [128, ph, pw, c] -> order cols (c, j): transpose to [128, c, ph, pw]
    w0 = w8[:, 0].transpose(0, 3, 1, 2).reshape(128, 128)
    w1 = w8[:, 1].transpose(0, 3, 1, 2).reshape(128, 128)
    return np.concatenate([w0 - w1, w1], axis=1)  # [128, 256]


def build_xstack(x):
    """[B, D, 128 rows=(od,oh,ow,cin), PLANE] shifted/padded copies of x."""
    xp = np.zeros((B, IN_C, D + 1, H + 1, W + 1), dtype=np.float32)
    xp[:, :, :D, :H, :W] = x
    S = np.empty((B, D, 2, 2, 2, IN_C, H, W), dtype=X_NP_DT)
    for od in range(2):
        for oh in range(2):
            for ow in range(2):
                # [B, cin, D, H, W] -> [B, D, cin, H, W]
                sl = xp[:, :, od:od + D, oh:oh + H, ow:ow + W]
                S[:, :, od, oh, ow] = sl.transpose(0, 2, 1, 3, 4).astype(X_NP_DT)
    return S.reshape(B, D, 128, PLANE)


def build_kernel(passes=1):
    from concourse import bass, bacc, mybir, tile

    f32 = mybir.dt.float32
    bf16 = mybir.dt.bfloat16
    x_dt = bf16 if X_NP_DT == ml_dtypes.bfloat16 else f32
    w_dt = bf16 if W_NP_DT == ml_dtypes.bfloat16 else f32
    Alu = mybir.AluOpType
    Act = mybir.ActivationFunctionType
    Ax = mybir.AxisListType

    nc = bacc.Bacc("TRN2", target_bir_lowering=False, debug=False,
                   num_devices=N_CORES)

    xs_h = nc.declare_dram_parameter("xs", [B_PER_CORE, D, 128, PLANE], x_dt,
                                     isOutput=False)
    wr_h = nc.declare_dram_parameter("wr", [128, 256], w_dt, isOutput=False)
    # bmm row: cols 0:512 = ones, 512:1024 = bias pattern (c,j) x4 blocks.
    # A K=1 matmul adds the conv bias into PSUM so no vector/pool engine
    # ever touches it.
    bmm_h = nc.declare_dram_parameter("bmm", [1, 1024], bf16, isOutput=False)
    sub_h = nc.declare_dram_parameter("subrep", [128, 1024], f32,
                                      isOutput=False)
    id_h = nc.declare_dram_parameter("ident", [128, 128], f32, isOutput=False)
    y_h = nc.declare_dram_parameter("y", [B_PER_CORE, D, PLANE], f32,
                                    isOutput=True)

    with tile.TileContext(nc) as tc:
        with (
            tc.tile_pool(name="const", bufs=1) as constp,
            tc.tile_pool(name="xslab", bufs=4) as xpool,
            tc.tile_pool(name="psum", bufs=4, space="PSUM") as psump,
            tc.tile_pool(name="tout", bufs=2) as toutp,
            tc.tile_pool(name="pooled", bufs=3) as plp,
            tc.tile_pool(name="pb", bufs=2) as pbp,
            tc.tile_pool(name="exp", bufs=2) as ep,
            tc.tile_pool(name="zr", bufs=2) as zp,
            tc.tile_pool(name="sm", bufs=3) as smp,
            tc.tile_pool(name="vv", bufs=3) as vp,
            tc.tile_pool(name="mm", bufs=3) as mmp,
            tc.tile_pool(name="ext", bufs=2) as extp,
            tc.tile_pool(name="sil", bufs=2) as silp,
            tc.tile_pool(name="ost", bufs=2) as ostp,
        ):
            wr = constp.tile([128, 256], w_dt)
            nc.sync.dma_start(wr[:], wr_h[:, :])
            bmm = constp.tile([1, 1024], bf16)
            nc.sync.dma_start(bmm[:], bmm_h[:, :])
            subrep = constp.tile([128, 1024], f32)
            nc.sync.dma_start(subrep[:], sub_h[:, :])
            ident = constp.tile([128, 128], f32)
            nc.sync.dma_start(ident[:], id_h[:, :])

            def emit_group(slab, pooled, g):
                """classmax for one 16-block group -> pooled[:, g]."""
                psum = psump.tile([128, GRP, 128], f32, tag="ps")
                for k in range(GRP):
                    blk = (g * GRP + k) * BLK
                    nc.tensor.matmul(
                        psum[:, k, :], slab[:, blk:blk + BLK], wr[:, 0:128],
                        start=True, stop=True)
                # pairmax = B + relu(D): relu in place (PE's has_written
                # bits survive the ScalarE write), then accumulate the
                # B-half matmuls on top.
                nc.scalar.activation(
                    psum[:].rearrange("p k c -> p (k c)"),
                    psum[:].rearrange("p k c -> p (k c)"), Act.Relu)
                for k in range(GRP):
                    blk = (g * GRP + k) * BLK
                    nc.tensor.matmul(
                        psum[:, k, :], slab[:, blk:blk + BLK], wr[:, 128:256],
                        start=False, stop=True, skip_group_check=True)
                # add the conv bias in PSUM: ones[128] (x) biasrow per bank
                for h in range(GRP // 4):
                    nc.tensor.matmul(
                        psum[:, 4 * h:4 * (h + 1), :].rearrange(
                            "p k c -> p (k c)"),
                        bmm[0:1, 0:128], bmm[0:1, 512:1024],
                        start=False, stop=True, skip_group_check=True)
                # pooled[c] = max over the 4 (ph,pw) pair-maxes (+bias)
                nc.vector.tensor_reduce(
                    pooled[:, g],
                    psum[:].rearrange("p k (c j) -> p k c j", c=32, j=4),
                    axis=Ax.X, op=Alu.max)

            def emit_tail_a(pend):
                """softmax numerator/denominator for a finished plane."""
                pooled, ext, col = pend[0], pend[1], pend[2]
                pl1 = pooled[:].rearrange("p g k c -> p (g k c)")
                E = ep.tile([128, 32, 32], f32, tag="E")
                nc.scalar.activation(
                    E[:].rearrange("p a b -> p (a b)"), pl1, Act.Exp)
                # Z = sum_c E: L1+L2+L3 on gpsimd, final reduce-4 on DVE
                e1 = mmp.tile([128, 32, 16], f32, tag="e1")
                nc.gpsimd.tensor_tensor(e1[:], E[:, :, 0:16], E[:, :, 16:32],
                                        Alu.add)
                e2 = mmp.tile([128, 32, 8], f32, tag="e2")
                nc.gpsimd.tensor_tensor(e2[:], e1[:, :, 0:8], e1[:, :, 8:16],
                                        Alu.add)
                e3 = mmp.tile([128, 32, 4], f32, tag="e3")
                nc.gpsimd.tensor_tensor(e3[:], e2[:, :, 0:4], e2[:, :, 4:8],
                                        Alu.add)
                Z = zp.tile([128, 32], f32, tag="Z")
                nc.vector.tensor_reduce(Z[:], e3[:], axis=Ax.X, op=Alu.add)
                R = zp.tile([128, 32], f32, tag="R")
                nc.vector.reciprocal(R[:], Z[:])
                return E, R

            def emit_tail_b(pend, E, R):
                """normalize, subtract, channel max/min extremes."""
                pooled, ext, col = pend[0], pend[1], pend[2]
                sm = smp.tile([128, 32, 32], f32, tag="sm")
                nc.gpsimd.tensor_tensor(
                    sm[:], E[:],
                    R[:].unsqueeze(2).broadcast_to([128, 32, 32]), Alu.mult)
                # v in bf16 so the DVE max/min trees run in 2x mode
                # (Pool TT does not support max/min per walrus codegen)
                v = vp.tile([128, 32, 32], bf16, tag="v")
                nc.gpsimd.tensor_tensor(
                    v[:], sm[:],
                    subrep[:].rearrange("p (a b) -> p a b", a=32, b=32),
                    Alu.subtract)
                m1 = mmp.tile([128, 32, 16], bf16, tag="m1")
                nc.vector.tensor_tensor(m1[:], v[:, :, 0:16], v[:, :, 16:32],
                                        Alu.max)
                m2 = mmp.tile([128, 32, 8], bf16, tag="m2")
                nc.vector.tensor_tensor(m2[:], m1[:, :, 0:8], m1[:, :, 8:16],
                                        Alu.max)
                nc.vector.tensor_reduce(
                    ext[:, 0, col:col + 32], m2[:], axis=Ax.X, op=Alu.max)
                # min(a,b) = (a+b) - max(a,b): L1 of the min tree rides Pool
                s1 = mmp.tile([128, 32, 16], f32, tag="s1")
                nc.gpsimd.tensor_tensor(s1[:], v[:, :, 0:16], v[:, :, 16:32],
                                        Alu.add)
                n1 = mmp.tile([128, 32, 16], bf16, tag="n1")
                nc.gpsimd.tensor_tensor(n1[:], s1[:], m1[:], Alu.subtract)
                n2 = mmp.tile([128, 32, 8], bf16, tag="n2")
                nc.vector.tensor_tensor(n2[:], n1[:, :, 0:8], n1[:, :, 8:16],
                                        Alu.min)
                nc.vector.tensor_reduce(
                    ext[:, 1, col:col + 32], n2[:], axis=Ax.X, op=Alu.min)

            def emit_b_final(b, ext):
                """silu on the per-b extremes, final max, output DMA."""
                sil = silp.tile([128, 2, D * BLKS_PER_PLANE], f32, tag="sil")
                nc.scalar.activation(
                    sil[:].rearrange("p a b -> p (a b)"),
                    ext[:].rearrange("p a b -> p (a b)"), Act.Silu)
                ost = ostp.tile([128, D * BLKS_PER_PLANE], f32, tag="ost")
                nc.vector.tensor_tensor(ost[:], sil[:, 0, :], sil[:, 1, :],
                                        Alu.max)
                # transpose ost on-chip so the output DMA is contiguous
                # (the AP-strided 4-byte scatter costs ~450us/pass on HW)
                tpfull = psump.tile([128, GRP, 128], f32, tag="ps")
                tp = tpfull[:, 0:4, :]
                for j in range(4):
                    nc.tensor.transpose(tp[:, j, :], ost[:, 128 * j:128 * (j + 1)],
                                        ident[:])
                T = toutp.tile([128, 4, 128], f32, tag="T")
                nc.scalar.activation(
                    T[:].rearrange("p a b -> p (a b)"),
                    tp[:].rearrange("p a b -> p (a b)"), Act.Copy)
                # T[r, j, p] = ost[p, 128j+r]; y flat = (128j+r)*128 + p
                nc.sync.dma_start(
                    y_h[b].flatten().rearrange("(j r p) -> r j p",
                                               j=4, r=BLK, p=BLK),
                    T[:])

            # Software-pipelined by one plane: the previous plane's tail is
            # interleaved between the current plane's two group reduces so
            # the DVE never waits on the Pool mult/sub chain.
            exts = []
            for b in range(B_PER_CORE):
                ext_b = extp.tile([128, 2, D * BLKS_PER_PLANE], f32,
                                  tag=f"ext{b}", name=f"ext{b}")
                exts.append(ext_b)
            pend = None
            for b_outer in range(passes * B_PER_CORE):
                b = b_outer % B_PER_CORE
                for d in range(D):
                    slab = xpool.tile([128, PLANE], x_dt, tag="slab")
                    # per-group DMA slices so the first matmuls start sooner
                    for g in range(GRPS_PER_PLANE):
                        c0 = g * GRP * BLK
                        nc.sync.dma_start(slab[:, c0:c0 + GRP * BLK],
                                          xs_h[b, d, :, c0:c0 + GRP * BLK])
                    pooled = plp.tile([128, GRPS_PER_PLANE, GRP, 32], f32,
                                      tag="pooled")
                    half = GRPS_PER_PLANE // 2
                    for g in range(half):
                        emit_group(slab, pooled, g)
                    if pend is not None:
                        ER = emit_tail_a(pend)
                    for g in range(half, GRPS_PER_PLANE):
                        emit_group(slab, pooled, g)
                    if pend is not None:
                        emit_tail_b(pend, *ER)
                        if passes > 1 and pend[3] == D - 1:
                            # multi-pass benchmark builds: keep the output
                            # path inside every pass so timing stays honest
                            emit_b_final(pend[4], pend[1])
                    pend = (pooled, exts[b], d * BLKS_PER_PLANE, d, b)
            # drain: last plane's tail, then per-b finals (the psum-slot
            # borrow for the output transpose is harmless in the drain region)
            ER = emit_tail_a(pend)
            if passes > 1:
                emit_tail_b(pend, *ER)
                emit_b_final(pend[4], pend[1])
            else:
                # interleave b0's final into the last tail's Pool-chain
                # window so the DVE drain has filler work
                emit_b_final(0, exts[0])
                emit_tail_b(pend, *ER)
                emit_b_final(1, exts[1])

    nc.compile()
    return nc


def _get_nc(passes=1):
    key = f"nc{passes}"
    if key not in _COMPILED:
        _COMPILED[key] = build_kernel(passes)
    return _COMPILED[key]


LAST_EXEC_NS = None


def build_in_maps(xs, wr, bias, subtract):
    # bmm: row0 = ones, row1 = bias at cols (c, j) tiled over 4 blocks
    bmm = np.concatenate([
        np.ones(512, np.float32),
        np.tile(np.repeat(bias.astype(np.float32), 4), 4),
    ]).reshape(1, 1024).astype(ml_dtypes.bfloat16)
    # subrep: [128, 1024] pattern (32 blocks x 32 ch), f32
    subrep = np.tile(subtract[None, None, :], (128, 32, 1)).reshape(
        128, 1024).astype(np.float32)

    in_maps = []
    for c in range(N_CORES):
        in_maps.append({
            "xs": np.ascontiguousarray(xs[c * B_PER_CORE:(c + 1) * B_PER_CORE]),
            "wr": wr,
            "bmm": bmm,
            "subrep": subrep,
            "ident": np.eye(128, dtype=np.float32),
        })
    return in_maps


def kernel(x, weight, bias, subtract):
    from concourse.bass_utils import run_bass_kernel_spmd

    x = np.asarray(x, dtype=np.float32)
    weight = np.asarray(weight, dtype=np.float32)
    bias = np.asarray(bias, dtype=np.float32)
    subtract = np.asarray(subtract, dtype=np.float32)

    nc = _get_nc()

    xs = build_xstack(x)                      # [B, D, 128, PLANE]
    wr = build_wrhs(weight).astype(W_NP_DT)   # [128, 256]
    in_maps = build_in_maps(xs, wr, bias, subtract)

    res = run_bass_kernel_spmd(nc, in_maps, core_ids=list(range(N_CORES)))
    outs = [res.results[c]["y"].reshape(B_PER_CORE, D, H, W)
            for c in range(N_CORES)]
    return np.concatenate(outs, axis=0)
